# revision 11
# baseline (speedup 1.0000x reference)
"""Trainium2 Bass kernel for nn_Attention_77927886618996.

Math (reference):
  y_t[n,h,l,r] = sum_f x[n,f,r] * T[h,l,f]        for T in {Q, K, D}
  t_n = y_t / ||y_t[n, :, :, :]||                  (norm over ALL heads, l, r)
  S[h,n,m] = sum_{l,r} q_n[n,h,l,r] * k_n[m,h,l,r]
  w = softmax_m(S);  v[n,h,l,r] = sum_m w[h,n,m] * d_n[m,h,l,r]
  out = v.reshape(n, h*l, r)

Sharding: one head per core (8 heads / 8 cores), x replicated. The per-n
norms couple all heads, so each core computes its head's partial sum of
squares and a tiny (3, 2048) AllReduce produces the global norms.

Per-core device program (head h == core id, fed via per-core weights):
  A)  W-stationary projections: psum[(q|k) l, n] += Wqk[f,:]^T @ xT[f,n]
      per rest-index r; same for D (M=64). Partial sums of squares via an
      indicator matmul over the squared activations.
  AR) AllReduce the (3, 2048) sums of squares -> 1/sqrt norms.
  A2) PE-transpose of y_d into (m, j) layout, scaled by 1/Nd[m].
  B)  Scores, transposed: S_T[m,n] = sum_lr yk[lr,m] * qn[lr,n]; softmax
      without max-subtraction (|S| <= 1 by Cauchy-Schwarz) as
      es[m,n] = exp(S_T * (1/Nk[m])) via ACT per-partition scale.
  C)  V^T[j,n] = sum_m dn[m,j] * es[m,n]  (PSUM accumulation over 16
      m-tiles), scaled by broadcast 1/Z[n] on evacuation.

kernel() is self-contained: hardcodes shapes, shards, runs, reassembles.
"""

import numpy as np
import ml_dtypes

N, F, R, H, L = 2048, 512, 8, 8, 64
NCORES = 8
FT = F // 128      # 4 f-tiles (contraction tiles for projections)
NCH = N // 512     # 4 column chunks of 512
NT = N // 128      # 16 m-tiles
JT = (L * R) // 128  # 4 (l,r)-tiles

BF16 = ml_dtypes.bfloat16

_CACHE = {}


def _build_nc():
    import concourse.bass as bass
    from concourse import bacc, mybir
    import concourse.tile as tile
    from contextlib import ExitStack

    bf = mybir.dt.bfloat16
    f32 = mybir.dt.float32

    nc = bacc.Bacc("TRN2", target_bir_lowering=False, debug=False,
                   num_devices=NCORES)

    xT = nc.dram_tensor("xT", [R, FT, 128, N], bf, kind="ExternalInput")
    wqk = nc.dram_tensor("wqk", [FT, 128, 128], bf, kind="ExternalInput")
    wd = nc.dram_tensor("wd", [FT, 128, L], bf, kind="ExternalInput")
    vout = nc.dram_tensor("vout", [JT * 128, N], f32, kind="ExternalOutput")

    ind_np = np.zeros((128, 2), BF16)
    ind_np[:64, 0] = 1
    ind_np[64:, 1] = 1
    ind_dram = nc.inline_tensor(ind_np, "indqk")
    ones64_dram = nc.inline_tensor(np.ones((64, 1), BF16), "ones64")
    ones1_dram = nc.inline_tensor(np.ones((1, 128), np.float32), "ones1")
    ones128_dram = nc.inline_tensor(np.ones((128, 1), BF16), "ones128")
    ident_dram = nc.inline_tensor(np.eye(128, dtype=BF16), "ident")

    with tile.TileContext(nc) as tc, ExitStack() as ctx:
        cpool = ctx.enter_context(tc.tile_pool(name="consts", bufs=1))
        ypool = ctx.enter_context(tc.tile_pool(name="ys", bufs=1))
        xpool = ctx.enter_context(tc.tile_pool(name="xs", bufs=2))
        sqpool = ctx.enter_context(tc.tile_pool(name="sqs", bufs=2))
        espool = ctx.enter_context(tc.tile_pool(name="es", bufs=1))
        smallpool = ctx.enter_context(tc.tile_pool(name="small", bufs=1))
        vpool = ctx.enter_context(tc.tile_pool(name="vstage", bufs=2))
        pspool = ctx.enter_context(
            tc.tile_pool(name="ps", bufs=2, space="PSUM"))
        drampool = ctx.enter_context(
            tc.tile_pool(name="dram", bufs=1, space="DRAM"))

        # ---- constants to SBUF
        ind_sb = cpool.tile([128, 2], bf, tag="ind")
        nc.sync.dma_start(ind_sb[:], ind_dram.ap())
        ones64_sb = cpool.tile([64, 1], bf, tag="ones64")
        nc.sync.dma_start(ones64_sb[:], ones64_dram.ap())
        ones1_sb = cpool.tile([1, 128], f32, tag="ones1")
        nc.sync.dma_start(ones1_sb[:], ones1_dram.ap())
        ident_sb = cpool.tile([128, 128], bf, tag="ident")
        nc.sync.dma_start(ident_sb[:], ident_dram.ap())
        wqk_sb = []
        wd_sb = []
        for ft in range(FT):
            t = cpool.tile([128, 128], bf, tag=f"wqk{ft}", name=f"wqks{ft}")
            nc.sync.dma_start(t[:], wqk[ft])
            wqk_sb.append(t)
            t = cpool.tile([128, L], bf, tag=f"wd{ft}", name=f"wds{ft}")
            nc.sync.dma_start(t[:], wd[ft])
            wd_sb.append(t)

        # ---- persistent activation arrays
        yq_sb = [ypool.tile([128, N], bf, tag=f"yq{t}", name=f"yq{t}")
                 for t in range(JT)]
        yk_sb = [ypool.tile([128, N], bf, tag=f"yk{t}", name=f"yk{t}")
                 for t in range(JT)]
        yd_sb = [ypool.tile([128, N], bf, tag=f"yd{t}", name=f"yd{t}")
                 for t in range(JT)]
        es_sb = [espool.tile([128, N], bf, tag=f"es{t}", name=f"es{t}")
                 for t in range(NT)]
        dn_sb = [ypool.tile([128, JT * 128], bf, tag=f"dn{t}", name=f"dn{t}")
                 for t in range(NT)]

        # sum-of-squares PSUM accumulators, one per n-chunk: rows 0-1 hold
        # q/k (indicator matmul), row 32 holds d (ones matmul at col grp 1)
        ssacc = [pspool.tile([33, 512], f32, tag=f"ssacc{i}", bufs=1,
                             name=f"ssacc{i}") for i in range(NCH)]

        # ---- stage A: projections + partial sums of squares
        for r in range(R):
            x_t = [xpool.tile([128, N], bf, tag=f"x{ft}", name=f"x{ft}")
                   for ft in range(FT)]
            for ft in range(FT):
                nc.sync.dma_start(x_t[ft][:], xT[r, ft])
            t = r // 2
            prow = (r % 2) * 64
            for nch in range(NCH):
                csl = slice(nch * 512, (nch + 1) * 512)
                psq = pspool.tile([128, 512], f32, tag="big")
                for ft in range(FT):
                    nc.tensor.matmul(psq[:], wqk_sb[ft][:], x_t[ft][:, csl],
                                     start=(ft == 0), stop=(ft == FT - 1))
                # unnormalized y_q / y_k, packed (r%2)*64+l on partitions
                nc.vector.tensor_copy(yq_sb[t][prow:prow + 64, csl],
                                      psq[0:64, :])
                nc.vector.tensor_copy(yk_sb[t][prow:prow + 64, csl],
                                      psq[64:128, :])
                sqq = sqpool.tile([128, 512], bf, tag="sqq")
                nc.scalar.square(sqq[:], psq[:])
                nc.tensor.matmul(ssacc[nch][0:2, :], ind_sb[:], sqq[:],
                                 start=(r == 0), stop=(r == R - 1),
                                 skip_group_check=True)

                psd = pspool.tile([64, 512], f32, tag="psd")
                for ft in range(FT):
                    nc.tensor.matmul(psd[:], wd_sb[ft][:], x_t[ft][:, csl],
                                     start=(ft == 0), stop=(ft == FT - 1))
                nc.vector.tensor_copy(yd_sb[t][prow:prow + 64, csl],
                                      psd[:])
                sqd = sqpool.tile([64, 512], bf, tag="sqd")
                nc.scalar.square(sqd[:], psd[:])
                nc.tensor.matmul(ssacc[nch][32:33, :], ones64_sb[:],
                                 sqd[:], tile_position=(0, 32),
                                 start=(r == 0), stop=(r == R - 1),
                                 skip_group_check=True)

        # ---- AllReduce the sums of squares across the 8 cores
        cc_in = drampool.tile([3, N], f32, tag="cc_in")
        cc_out = drampool.tile([3, N], f32, tag="cc_out")
        ss_sb = smallpool.tile([33, N], f32, tag="ss_sb")
        for nch in range(NCH):
            csl = slice(nch * 512, (nch + 1) * 512)
            nc.vector.tensor_copy(ss_sb[0:2, csl], ssacc[nch][0:2, :])
            nc.vector.tensor_copy(ss_sb[32:33, csl], ssacc[nch][32:33, :])
        nc.sync.dma_start(cc_in[0:2, :], ss_sb[0:2, :])
        nc.sync.dma_start(cc_in[2:3, :], ss_sb[32:33, :])
        nc.gpsimd.collective_compute(
            "AllReduce",
            mybir.AluOpType.add,
            replica_groups=[list(range(NCORES))],
            ins=[cc_in.opt()],
            outs=[cc_out.opt()],
        )

        # ---- norms: rq as a row (for q), rk/rd as (128, NT) columns
        rq_row = smallpool.tile([1, N], f32, tag="rq_row")
        nc.sync.dma_start(rq_row[:], cc_out[0:1, :])
        rk_cols = smallpool.tile([128, NT], f32, tag="rk_cols")
        nc.sync.dma_start(rk_cols[:],
                          cc_out[1:2, :].rearrange("a (t p) -> (a p) t", p=128))
        rd_cols = smallpool.tile([128, NT], f32, tag="rd_cols")
        nc.sync.dma_start(rd_cols[:],
                          cc_out[2:3, :].rearrange("a (t p) -> (a p) t", p=128))
        # in-place sqrt then reciprocal -> 1/norm
        for t_ in (rq_row, rk_cols, rd_cols):
            nc.scalar.sqrt(t_[:], t_[:])
            nc.vector.reciprocal(t_[:], t_[:])

        # ---- broadcast 1/Nq over partitions (K=1 outer-product matmul)
        rnqb = smallpool.tile([128, N], bf, tag="rnqb")
        for nch in range(NCH):
            csl = slice(nch * 512, (nch + 1) * 512)
            bps = pspool.tile([128, 512], f32, tag="big")
            nc.tensor.matmul(bps[:], ones1_sb[:], rq_row[:, csl],
                             start=True, stop=True)
            nc.vector.tensor_copy(rnqb[:, csl], bps[:])
        # normalize q in place
        for t in range(JT):
            nc.vector.tensor_mul(yq_sb[t][:], yq_sb[t][:], rnqb[:])

        # ---- A2: transpose y_d -> dn[m, j], scaling by 1/Nd[m]
        for mt in range(NT):
            msl = slice(mt * 128, (mt + 1) * 128)
            for jt in range(JT):
                tp = pspool.tile([128, 128], bf, tag="psd")
                nc.tensor.transpose(tp[:], yd_sb[jt][:, msl], ident_sb[:])
                nc.vector.tensor_scalar_mul(
                    dn_sb[mt][:, jt * 128:(jt + 1) * 128], tp[:],
                    rd_cols[:, mt:mt + 1])

        # ---- stage B: es[m,n] = exp(S_T * 1/Nk[m]); Z via ones-matmul
        ones128_sb = cpool.tile([128, 1], bf, tag="ones128")
        nc.sync.dma_start(ones128_sb[:], ones128_dram.ap())
        zps = [pspool.tile([1, 512], f32, tag=f"ssacc{i}", bufs=1,
                           name=f"zps{i}") for i in range(NCH)]
        for mt in range(NT):
            msl = slice(mt * 128, (mt + 1) * 128)
            for nch in range(NCH):
                csl = slice(nch * 512, (nch + 1) * 512)
                sps = pspool.tile([128, 512], f32, tag="big")
                for lt in range(JT):
                    nc.tensor.matmul(sps[:], yk_sb[lt][:, msl],
                                     yq_sb[lt][:, csl],
                                     start=(lt == 0), stop=(lt == JT - 1))
                nc.scalar.activation(es_sb[mt][:, csl], sps[:],
                                     mybir.ActivationFunctionType.Exp,
                                     bias=0.0, scale=rk_cols[:, mt:mt + 1])
                nc.tensor.matmul(zps[nch][:], ones128_sb[:],
                                 es_sb[mt][:, csl],
                                 start=(mt == 0), stop=(mt == NT - 1))

        # 1/Z as a row, then broadcast over partitions via K=1 matmul
        rz_row = smallpool.tile([1, N], f32, tag="rz_row")
        for nch in range(NCH):
            csl = slice(nch * 512, (nch + 1) * 512)
            nc.vector.reciprocal(rz_row[:, csl], zps[nch][:])
        rzb = smallpool.tile([128, N], f32, tag="rzb")
        for nch in range(NCH):
            csl = slice(nch * 512, (nch + 1) * 512)
            bps = pspool.tile([128, 512], f32, tag="big")
            nc.tensor.matmul(bps[:], ones1_sb[:], rz_row[:, csl],
                             start=True, stop=True)
            nc.vector.tensor_copy(rzb[:, csl], bps[:])

        # ---- stage C: V^T[j,n] accumulated over m-tiles, scaled by 1/Z
        for nch in range(NCH):
            csl = slice(nch * 512, (nch + 1) * 512)
            for jt in range(JT):
                vps = pspool.tile([128, 512], f32, tag="big")
                for mt in range(NT):
                    nc.tensor.matmul(vps[:],
                                     dn_sb[mt][:, jt * 128:(jt + 1) * 128],
                                     es_sb[mt][:, csl],
                                     start=(mt == 0), stop=(mt == NT - 1))
                vst = vpool.tile([128, 512], f32, tag="vst")
                nc.vector.tensor_mul(vst[:], vps[:], rzb[:, csl])
                nc.sync.dma_start(vout[jt * 128:(jt + 1) * 128, csl], vst[:])

    nc.compile()
    return nc


def _get_nc():
    if "nc" not in _CACHE:
        _CACHE["nc"] = _build_nc()
    return _CACHE["nc"]


def _prep_inputs(x, Q, K, D):
    """Host-side shard prep. Returns per-core input maps."""
    x = np.asarray(x, dtype=np.float32)
    Q = np.asarray(Q, dtype=np.float32)
    K = np.asarray(K, dtype=np.float32)
    D = np.asarray(D, dtype=np.float32)
    # xT[r, ft, fp, n] = x[n, 128*ft+fp, r]
    xT = np.ascontiguousarray(x.transpose(2, 1, 0)).reshape(R, FT, 128, N)
    xT = xT.astype(BF16)
    in_maps = []
    for c in range(NCORES):
        wqk = np.concatenate([Q[c], K[c]], axis=0).T  # (F, 128)
        wqk = np.ascontiguousarray(wqk).reshape(FT, 128, 128).astype(BF16)
        wd = np.ascontiguousarray(D[c].T).reshape(FT, 128, L).astype(BF16)
        in_maps.append({"xT": xT, "wqk": wqk, "wd": wd})
    return in_maps


def _assemble(results):
    """Per-core (512, 2048) V^T -> full (N, H*L, R) output."""
    out = np.empty((N, H * L, R), dtype=np.float32)
    for c in range(NCORES):
        vT = results[c]["vout"]  # (JT*128, N): row j = jt*128 + p,
        # p = (r%2)*64 + l, r = 2*jt + p//64
        oc = vT.reshape(JT, 2, 64, N)          # [jt, rhalf, l, n]
        out[:, c * L:(c + 1) * L, :] = oc.transpose(3, 2, 0, 1).reshape(
            N, L, R)
    return out


def kernel(x, Q, K, D, _trace=False):
    from concourse.bass_utils import run_bass_kernel_spmd

    nc = _get_nc()
    in_maps = _prep_inputs(x, Q, K, D)
    res = run_bass_kernel_spmd(nc, in_maps, core_ids=list(range(NCORES)),
                               trace=_trace)
    out = _assemble(res.results)
    if _trace:
        _CACHE["last_results"] = res
    return out
